# revision 4
# baseline (speedup 1.0000x reference)
"""L2 ECE loss (15-bin histogram binning) on 8 Trainium2 NeuronCores.

Strategy (data-parallel, matches the sharding hint):
  - Shard the N=2^25 element axis contiguously across 8 cores (2^22 each).
  - Per core, stream [128, F] fp32 tiles of confidences/accuracies.
    For each of the 15 bin boundaries t_j (exact f32 values of
    jnp.linspace(0,1,16)), compute in fused single passes:
      * ACT engine:  A_j   = sum(sign(c - t_j))        (fused accum_out)
      * DVE engine:  Td_j  = sum((c > t_j) * (c - a))  (fused accum_out)
    Cumulative counts T_j = (A_j + N)/2; per-bin counts and per-bin
    sum(c-a) follow by adjacent differences.  ECE = sum_b D_b^2/(cnt_b*N).
  - Per-core partial accumulators ([128, ntiles] slots) are DMA'd out and
    the tiny final reduction (3 KB of data) is done on the host in f64.
"""

import numpy as np

import concourse.bass as bass
import concourse.tile as tile
from concourse import bacc, mybir
from concourse import bass_utils

# -- problem constants (hardcoded per spec) ---------------------------------
N_TOTAL = 33554432  # 2**25
N_CORES = 8
NC_PER = N_TOTAL // N_CORES  # 4194304 per core
P = 128
F = 4096  # free-dim elements per tile
T = NC_PER // (P * F)  # 8 tiles per core
N_BINS = 15

# f32 bit patterns of jnp.linspace(0.0, 1.0, 16) — must match the reference
# bit-exactly (jnp.linspace rounds differently from np.linspace).
_BOUND_BITS = [
    0x00000000, 0x3D888889, 0x3E088889, 0x3E4CCCCE,
    0x3E888889, 0x3EAAAAAB, 0x3ECCCCCE, 0x3EEEEEF0,
    0x3F088889, 0x3F19999A, 0x3F2AAAAB, 0x3F3BBBBC,
    0x3F4CCCCE, 0x3F5DDDDF, 0x3F6EEEF0, 0x3F800000,
]
BOUNDS = np.array(_BOUND_BITS, dtype=np.uint32).view(np.float32)

_CACHE = {}
TRACE = False
LAST_RESULTS = None


def _build(repeat=1):
    f32 = mybir.dt.float32
    nc = bacc.Bacc(
        "TRN2",
        target_bir_lowering=False,
        debug=False,
        enable_asserts=False,
        num_devices=N_CORES,
    )
    # pre-register activation bias constants (-t_j) as const APs
    for j in range(N_BINS):
        val = -float(BOUNDS[j])
        if (f32, val) not in nc.const_aps.aps:
            t = nc.alloc_sbuf_tensor(f"const-bias-{j}", [128, 1], f32)
            nc.gpsimd.memset(t.ap(), val)
            nc.const_aps.aps[(f32, val)] = t.ap()
    nc.all_engine_barrier()

    conf = nc.dram_tensor("conf", [NC_PER], f32, kind="ExternalInput").ap()
    acc = nc.dram_tensor("acc", [NC_PER], f32, kind="ExternalInput").ap()
    # accumulator slot layouts: DVE -> [P, T*16] (j=0..14 used),
    # ACT -> [P, T*16] (j=0..14 used)
    out_dve = nc.dram_tensor("out_dve", [P, T * 16], f32, kind="ExternalOutput").ap()
    out_act = nc.dram_tensor("out_act", [P, T * 16], f32, kind="ExternalOutput").ap()

    conf_t = conf.rearrange("(t p f) -> t p f", p=P, f=F)
    acc_t = acc.rearrange("(t p f) -> t p f", p=P, f=F)

    with tile.TileContext(nc) as tc:
        with (
            tc.tile_pool(name="io", bufs=3) as io_pool,
            tc.tile_pool(name="work", bufs=2) as work_pool,
            tc.tile_pool(name="scr", bufs=1) as scr_pool,
            tc.tile_pool(name="accs", bufs=1) as acc_pool,
        ):
            dve_scr = scr_pool.tile([P, F], f32, tag="dve_scr")
            act_scr = scr_pool.tile([P, F], f32, tag="act_scr")
            acc_dve = acc_pool.tile([P, T * 16], f32, tag="acc_dve")
            acc_act = acc_pool.tile([P, T * 16], f32, tag="acc_act")

            for t in range(T * repeat):
                t = t % T
                c = io_pool.tile([P, F], f32, tag="c")
                nc.sync.dma_start(c[:], conf_t[t])
                a = io_pool.tile([P, F], f32, tag="a")
                nc.sync.dma_start(a[:], acc_t[t])

                # d = c - a  (fp32)
                d = work_pool.tile([P, F], f32, tag="d")
                nc.vector.scalar_tensor_tensor(
                    out=d[:],
                    in0=c[:],
                    scalar=0.0,
                    in1=a[:],
                    op0=mybir.AluOpType.bypass,
                    op1=mybir.AluOpType.subtract,
                )

                # DVE: Td_j = sum((c > t_j) * d), fused accumulate
                for j in range(N_BINS):
                    nc.vector.scalar_tensor_tensor(
                        out=dve_scr[:],
                        in0=c[:],
                        scalar=float(BOUNDS[j]),
                        in1=d[:],
                        op0=mybir.AluOpType.is_gt,
                        op1=mybir.AluOpType.mult,
                        accum_out=acc_dve[:, t * 16 + j : t * 16 + j + 1],
                    )

                # ACT: A_j = sum(sign(c - t_j)), fused accumulate
                for j in range(N_BINS):
                    nc.scalar.activation(
                        out=act_scr[:],
                        in_=c[:],
                        func=mybir.ActivationFunctionType.Sign,
                        bias=-float(BOUNDS[j]),
                        scale=1.0,
                        accum_out=acc_act[:, t * 16 + j : t * 16 + j + 1],
                    )

            nc.sync.dma_start(out_dve[:], acc_dve[:])
            nc.sync.dma_start(out_act[:], acc_act[:])

    nc.compile()
    return nc


def kernel(confidences, accuracies):
    global LAST_RESULTS
    conf = np.ascontiguousarray(np.asarray(confidences, dtype=np.float32))
    accu = np.ascontiguousarray(np.asarray(accuracies, dtype=np.float32))
    assert conf.shape == (N_TOTAL,) and accu.shape == (N_TOTAL,)

    if "nc" not in _CACHE:
        _CACHE["nc"] = _build()
    nc = _CACHE["nc"]

    conf_sh = conf.reshape(N_CORES, NC_PER)
    accu_sh = accu.reshape(N_CORES, NC_PER)
    in_maps = [
        {"conf": conf_sh[i], "acc": accu_sh[i]} for i in range(N_CORES)
    ]
    res = bass_utils.run_bass_kernel_spmd(
        nc, in_maps, core_ids=list(range(N_CORES)), trace=TRACE
    )
    LAST_RESULTS = res

    # host-side finish (tiny): combine per-core partial sums in f64
    Td = np.zeros(N_BINS + 1, dtype=np.float64)  # cumulative sum(d) above t_j
    A = np.zeros(N_BINS + 1, dtype=np.float64)  # cumulative sum(sign)
    for r in res.results:
        od = np.asarray(r["out_dve"], dtype=np.float64).reshape(P, T, 16)
        oa = np.asarray(r["out_act"], dtype=np.float64).reshape(P, T, 16)
        Td[:N_BINS] += od.sum(axis=(0, 1))[:N_BINS]
        A[:N_BINS] += oa.sum(axis=(0, 1))[:N_BINS]

    Tcnt = (A[:N_BINS] + N_TOTAL) / 2.0  # counts of {c > t_j}
    Tcnt = np.concatenate([Tcnt, [0.0]])
    Td[N_BINS] = 0.0

    cnt = Tcnt[:N_BINS] - Tcnt[1:]  # per-bin counts
    D = Td[:N_BINS] - Td[1:]  # per-bin sum(c - a)
    with np.errstate(divide="ignore", invalid="ignore"):
        terms = np.where(cnt > 0.5, D * D / np.maximum(cnt, 1.0) / N_TOTAL, 0.0)
    return np.float32(terms.sum())
